# revision 9
# baseline (speedup 1.0000x reference)
"""DLinear (causal sliding-window-mean decomposition + two linear heads) on 8 TRN2 NeuronCores.

Math: out = trend @ tW.T + seasonal @ sW.T + (tb + sb), seasonal = x - trend,
trend[:, j] = mean(x[:, max(0, j-24):j+1]) (window 25, causal).

trend is linear in x: trend = x @ A with A[i, j] = 1/c(j) for j-24 <= i <= j,
c(j) = min(j+1, 25). Folding:
    out = x @ W_eff + (tb + sb),  W_eff = sW.T + A @ (tW - sW).T
so the sliding-window work lands on the small [720, 2048] weight delta instead
of x, and the x-side is a single [B, S] @ [S, O] matmul.

Sharding: 2D, 4-way batch x 2-way output: core i handles batch rows
[1024*(i%4), ...) and output columns [360*(i//4), ...). Each core builds its
W_eff half via banded matmuls on the TensorE (b0/b1 carry the 1/count
normalization); halving O per core halves the W-prep and lets the W-prep PSUM
tile fit one bank, so it double-buffers alongside the 6 out-accumulator banks
(8 total). No collectives.

Device layout: the contraction (S) must sit on SBUF partitions for the
TensorE, so the host passes x.T slices and transposed weight halves packed
block-interleaved - layout/dtype prep only, all arithmetic is on device.
Inputs are fed as fp16: its 11-bit mantissa matches the TensorE's own
single-pass fp32 (f32r) rounding, so accuracy stays at the few-1e-4 level
(measured) while DMA bytes halve; accumulation is fp32 in PSUM throughout.
x streams on the ScalarE HWDGE queues, weights on the SyncE queues, so
neither load delays the other.
"""

import sys

sys.path.insert(0, "/opt/trn_rl_repo")

import numpy as np

import concourse.bacc as bacc
import concourse.mybir as mybir
from concourse.tile import TileContext
from concourse.bass_utils import run_bass_kernel_spmd

B, S, O = 4096, 2048, 720
WIN = 25
NCORES = 8
NBG, NOG = 4, 2           # batch groups x output groups
BC = B // NBG             # 1024 batch rows per core
OC = O // NOG             # 360 output cols per core
NK = S // 128             # 16 S-blocks of 128
WBLK = 2 * OC             # 720 packed weight cols per S-block (tW-half || sW-half)

F32 = mybir.dt.float32
F16 = mybir.dt.float16

_nc_cache = None


def _build_bands():
    """Band matrices as matmul lhsT ([K=j, M=i]): G^T[i,o] = sum_j band[j,i] * D^T[j,o].

    b0f: within-block band for S-block 0, carries 1/c(j) = 1/min(j+1, 25).
    b0r: within-block band for blocks >= 1, carries 1/25.
    b1:  next-block band (rows j2 of block k+1 contribute to i >= 104+j2), 1/25.
    """
    b0f = np.zeros((128, 128), np.float32)
    b0r = np.zeros((128, 128), np.float32)
    b1 = np.zeros((128, 128), np.float32)
    for i in range(128):
        for j in range(i, min(i + WIN, 128)):
            b0f[j, i] = 1.0 / min(j + 1, WIN)
            b0r[j, i] = 1.0 / WIN
        for j2 in range(0, i - 104 + 1):
            b1[j2, i] = 1.0 / WIN
    return b0f.astype(np.float16), b0r.astype(np.float16), b1.astype(np.float16)


def build_in_maps(x, trend_W, trend_b, seasonal_W, seasonal_b):
    """Host-side shard + layout/dtype prep. Returns per-core input dicts."""
    x16 = np.asarray(x, dtype=np.float16)
    tT = np.asarray(trend_W, dtype=np.float16).T      # [S, O]
    sT = np.asarray(seasonal_W, dtype=np.float16).T
    bs = (np.asarray(trend_b, np.float32) + np.asarray(seasonal_b, np.float32)).reshape(O, 1)
    b0f, b0r, b1 = _build_bands()

    # weight packs per output half: [128, NK*720], block k = tT half || sT half
    wpks = []
    for g in range(NOG):
        tR = tT[:, g * OC : (g + 1) * OC].reshape(NK, 128, OC)
        sR = sT[:, g * OC : (g + 1) * OC].reshape(NK, 128, OC)
        wpks.append(
            np.ascontiguousarray(
                np.concatenate([tR, sR], axis=2).transpose(1, 0, 2).reshape(128, NK * WBLK)
            )
        )
    bsums = [np.ascontiguousarray(bs[g * OC : (g + 1) * OC]) for g in range(NOG)]

    xT = x16.T  # [S, B] view
    xpks = []
    for c in range(NBG):
        xc = np.ascontiguousarray(xT[:, c * BC : (c + 1) * BC])  # [S, BC]
        xpks.append(
            np.ascontiguousarray(
                xc.reshape(NK, 128, BC).transpose(1, 0, 2).reshape(128, NK * BC)
            )
        )

    in_maps = []
    for i in range(NCORES):
        g, c = i // NBG, i % NBG
        in_maps.append(
            {"xpk": xpks[c], "wpk": wpks[g], "bsum": bsums[g],
             "b0f": b0f, "b0r": b0r, "b1": b1}
        )
    return in_maps


def _build_nc():
    nc = bacc.Bacc()
    xpk = nc.declare_dram_parameter("xpk", [128, NK * BC], F16, isOutput=False)
    wpk = nc.declare_dram_parameter("wpk", [128, NK * WBLK], F16, isOutput=False)
    bsum = nc.declare_dram_parameter("bsum", [OC, 1], F32, isOutput=False)
    b0f = nc.declare_dram_parameter("b0f", [128, 128], F16, isOutput=False)
    b0r = nc.declare_dram_parameter("b0r", [128, 128], F16, isOutput=False)
    b1 = nc.declare_dram_parameter("b1", [128, 128], F16, isOutput=False)
    outT = nc.declare_dram_parameter("outT", [OC, BC], F32, isOutput=True)

    OTS = [(0, 128), (128, 128), (256, OC - 256)]   # o-tiles within the 360 half
    NBH = BC // 512                                  # 2 batch halves (psum N=512)

    with TileContext(nc) as tc:
        with (
            tc.tile_pool(name="consts", bufs=1) as consts,
            tc.tile_pool(name="wlp", bufs=3) as wlp,
            tc.tile_pool(name="xcp", bufs=8) as xcp,
            tc.tile_pool(name="dp", bufs=3) as dp,
            tc.tile_pool(name="wp", bufs=1) as wp,
            tc.tile_pool(name="op", bufs=3) as op,
            tc.tile_pool(name="pw", bufs=2, space="PSUM") as pwp,
            tc.tile_pool(name="po", bufs=1, space="PSUM") as pop,
        ):
            # constants (SyncE queue)
            b0f_t = consts.tile([128, 128], F16, tag="b0f")
            b0r_t = consts.tile([128, 128], F16, tag="b0r")
            b1_t = consts.tile([128, 128], F16, tag="b1")
            nc.sync.dma_start(out=b0f_t[:], in_=b0f[:])
            nc.sync.dma_start(out=b0r_t[:], in_=b0r[:])
            nc.sync.dma_start(out=b1_t[:], in_=b1[:])
            bs_t = []
            for ot, (o0, ow) in enumerate(OTS):
                t = consts.tile([128, 1], F32, tag=f"bs{ot}", name=f"bs{ot}")
                nc.sync.dma_start(out=t[0:ow, :], in_=bsum[o0 : o0 + ow, :])
                bs_t.append(t)

            # weight blocks: 8 chunks x 2 S-blocks on SyncE
            wl_t = {}
            for c in range(NK // 2):
                wl_t[c] = wlp.tile([128, 2 * WBLK], F16, tag="wl", name=f"wl{c}")
                nc.sync.dma_start(
                    out=wl_t[c][:], in_=wpk[:, 2 * WBLK * c : 2 * WBLK * (c + 1)]
                )
            # x: 8 chunks x 2 S-blocks on ScalarE queues (parallel to weights)
            xc_t = {}
            for c in range(NK // 2):
                xc_t[c] = xcp.tile([128, 2 * BC], F16, tag="xc", name=f"xc{c}")
                nc.scalar.dma_start(
                    out=xc_t[c][:], in_=xpk[:, 2 * BC * c : 2 * BC * (c + 1)]
                )

            # psum: 6 out accumulators (1 bank each) + double-buffered W-prep bank
            po_t = {}
            for ot in range(len(OTS)):
                for h in range(NBH):
                    po_t[ot, h] = pop.tile(
                        [128, 512], F32, tag=f"po{ot}_{h}", name=f"po{ot}_{h}"
                    )

            def wslice(k, which):  # tW half (0) or sW half (1) of S-block k
                base = (k % 2) * WBLK + which * OC
                return wl_t[k // 2][:, base : base + OC]

            def xslice(k, h):
                return xc_t[k // 2][:, (k % 2) * BC + 512 * h : (k % 2) * BC + 512 * (h + 1)]

            # ---- Phase A: build all W_eff blocks (kept resident, 16 x 90KB fp16)
            d_t, w_t = {}, {}
            for k in range(NK + 1):
                if k < NK:
                    d_t[k] = dp.tile([128, OC], F16, tag="d", name=f"d{k}")
                    nc.vector.tensor_tensor(
                        out=d_t[k][:], in0=wslice(k, 0), in1=wslice(k, 1),
                        op=mybir.AluOpType.subtract,
                    )
                if k >= 1:
                    j = k - 1
                    # banded matmuls: pw = b0 @ D_j (+ b1 @ D_{j+1})
                    pw = pwp.tile([128, OC], F32, tag="pw", name=f"pw{j}")
                    b0 = b0f_t if j == 0 else b0r_t
                    last = j == NK - 1
                    nc.tensor.matmul(pw[:], b0[:], d_t[j][:], start=True, stop=last)
                    if not last:
                        nc.tensor.matmul(pw[:], b1_t[:], d_t[j + 1][:], start=False, stop=True)
                    # W_eff^T_j = pw + sWT_j   (PSUM evac on DVE)
                    w_t[j] = wp.tile([128, OC], F16, tag=f"w{j}", name=f"w{j}")
                    nc.vector.tensor_tensor(
                        out=w_t[j][:], in0=pw[:], in1=wslice(j, 1), op=mybir.AluOpType.add
                    )
            # ---- Phase B: one dense, wait-free PE stream of 96 accumulating matmuls
            for k in range(NK):
                for ot, (o0, ow) in enumerate(OTS):
                    for h in range(NBH):
                        nc.tensor.matmul(
                            po_t[ot, h][0:ow, :],
                            w_t[k][:, o0 : o0 + ow],
                            xslice(k, h),
                            start=(k == 0),
                            stop=(k == NK - 1),
                        )
            # epilogue: bias add fused into PSUM evac on ScalarE, then store
            # (stores split across the two HWDGE queues)
            for ot, (o0, ow) in enumerate(OTS):
                for h in range(NBH):
                    osb = op.tile([128, 512], F32, tag="o", name=f"osb{ot}_{h}")
                    nc.scalar.activation(
                        out=osb[0:ow, :], in_=po_t[ot, h][0:ow, :],
                        func=mybir.ActivationFunctionType.Identity,
                        bias=bs_t[ot][0:ow, :],
                    )
                    dma = nc.sync if h == 0 else nc.scalar
                    dma.dma_start(
                        out=outT[o0 : o0 + ow, 512 * h : 512 * (h + 1)],
                        in_=osb[0:ow, :],
                    )

    nc.compile()
    return nc


def kernel(x, trend_W, trend_b, seasonal_W, seasonal_b):
    global _nc_cache
    if _nc_cache is None:
        _nc_cache = _build_nc()
    in_maps = build_in_maps(x, trend_W, trend_b, seasonal_W, seasonal_b)
    res = run_bass_kernel_spmd(_nc_cache, in_maps, list(range(NCORES)))
    full = np.empty((O, B), np.float32)
    for i, r in enumerate(res.results):
        g, c = i // NBG, i % NBG
        full[g * OC : (g + 1) * OC, c * BC : (c + 1) * BC] = r["outT"]
    return np.ascontiguousarray(full.T)


# revision 11
# speedup vs baseline: 1.0960x; 1.0960x over previous
"""DLinear (causal sliding-window-mean decomposition + two linear heads) on 8 TRN2 NeuronCores.

Math: out = trend @ tW.T + seasonal @ sW.T + (tb + sb), seasonal = x - trend,
trend[:, j] = mean(x[:, max(0, j-24):j+1]) (window 25, causal).

trend is linear in x: trend = x @ A with A[i, j] = 1/c(j) for j-24 <= i <= j,
c(j) = min(j+1, 25). Folding:
    out = x @ W_eff + (tb + sb),  W_eff = sW.T + A @ (tW - sW).T
so the sliding-window work lands on the small [720, 2048] weight delta instead
of x, and the x-side is a single [B, S] @ [S, O] matmul.

Sharding: 2D, 4-way batch x 2-way output: core i handles batch rows
[1024*(i%4), ...) and output columns [360*(i//4), ...). Each core builds its
W_eff half via banded matmuls on the TensorE (b0/b1 carry the 1/count
normalization); halving O per core halves the W-prep and lets the W-prep PSUM
tile fit one bank, so it double-buffers alongside the 6 out-accumulator banks
(8 total). No collectives.

Device layout: the contraction (S) must sit on SBUF partitions for the
TensorE, so the host passes x.T slices and transposed weight halves packed
block-interleaved - layout/dtype prep only, all arithmetic is on device.
Inputs are fed as fp16: its 11-bit mantissa matches the TensorE's own
single-pass fp32 (f32r) rounding, so accuracy stays at the few-1e-4 level
(measured) while DMA bytes halve; accumulation is fp32 in PSUM throughout.
x streams on the ScalarE HWDGE queues, weights on the SyncE queues, so
neither load delays the other.
"""

import sys

sys.path.insert(0, "/opt/trn_rl_repo")

import numpy as np

import concourse.bacc as bacc
import concourse.mybir as mybir
from concourse.tile import TileContext
from concourse.bass_utils import run_bass_kernel_spmd

B, S, O = 4096, 2048, 720
WIN = 25
NCORES = 8
NBG, NOG = 4, 2           # batch groups x output groups
BC = B // NBG             # 1024 batch rows per core
OC = O // NOG             # 360 output cols per core
NK = S // 128             # 16 S-blocks of 128
WBLK = 2 * OC             # 720 packed weight cols per S-block (tW-half || sW-half)

F32 = mybir.dt.float32
F16 = mybir.dt.float16

_nc_cache = None


def _build_bands():
    """Band matrices as matmul lhsT ([K=j, M=i]): G^T[i,o] = sum_j band[j,i] * D^T[j,o].

    b0f: within-block band for S-block 0, carries 1/c(j) = 1/min(j+1, 25).
    b0r: within-block band for blocks >= 1, carries 1/25.
    b1:  next-block band (rows j2 of block k+1 contribute to i >= 104+j2), 1/25.
    """
    b0f = np.zeros((128, 128), np.float32)
    b0r = np.zeros((128, 128), np.float32)
    b1 = np.zeros((128, 128), np.float32)
    for i in range(128):
        for j in range(i, min(i + WIN, 128)):
            b0f[j, i] = 1.0 / min(j + 1, WIN)
            b0r[j, i] = 1.0 / WIN
        for j2 in range(0, i - 104 + 1):
            b1[j2, i] = 1.0 / WIN
    return b0f.astype(np.float16), b0r.astype(np.float16), b1.astype(np.float16)


def build_in_maps(x, trend_W, trend_b, seasonal_W, seasonal_b):
    """Host-side shard + layout/dtype prep. Returns per-core input dicts."""
    x16 = np.asarray(x, dtype=np.float16)
    tT = np.asarray(trend_W, dtype=np.float16).T      # [S, O]
    sT = np.asarray(seasonal_W, dtype=np.float16).T
    bs = (np.asarray(trend_b, np.float32) + np.asarray(seasonal_b, np.float32)).reshape(O, 1)
    b0f, b0r, b1 = _build_bands()

    # weight packs per output half: [128, NK*720], block k = tT half || sT half
    wpks = []
    for g in range(NOG):
        tR = tT[:, g * OC : (g + 1) * OC].reshape(NK, 128, OC)
        sR = sT[:, g * OC : (g + 1) * OC].reshape(NK, 128, OC)
        wpks.append(
            np.ascontiguousarray(
                np.concatenate([tR, sR], axis=2).transpose(1, 0, 2).reshape(128, NK * WBLK)
            )
        )
    bsums = [np.ascontiguousarray(bs[g * OC : (g + 1) * OC]) for g in range(NOG)]

    xT = x16.T  # [S, B] view
    xpks = []
    for c in range(NBG):
        xc = np.ascontiguousarray(xT[:, c * BC : (c + 1) * BC])  # [S, BC]
        xpks.append(
            np.ascontiguousarray(
                xc.reshape(NK, 128, BC).transpose(1, 0, 2).reshape(128, NK * BC)
            )
        )

    in_maps = []
    for i in range(NCORES):
        g, c = i // NBG, i % NBG
        in_maps.append(
            {"xpk": xpks[c], "wpk": wpks[g], "bsum": bsums[g],
             "b0f": b0f, "b0r": b0r, "b1": b1}
        )
    return in_maps


def _build_nc():
    nc = bacc.Bacc()
    xpk = nc.declare_dram_parameter("xpk", [128, NK * BC], F16, isOutput=False)
    wpk = nc.declare_dram_parameter("wpk", [128, NK * WBLK], F16, isOutput=False)
    bsum = nc.declare_dram_parameter("bsum", [OC, 1], F32, isOutput=False)
    b0f = nc.declare_dram_parameter("b0f", [128, 128], F16, isOutput=False)
    b0r = nc.declare_dram_parameter("b0r", [128, 128], F16, isOutput=False)
    b1 = nc.declare_dram_parameter("b1", [128, 128], F16, isOutput=False)
    outT = nc.declare_dram_parameter("outT", [OC, BC], F32, isOutput=True)

    OTS = [(0, 128), (128, 128), (256, OC - 256)]   # o-tiles within the 360 half
    NBH = BC // 512                                  # 2 batch halves (psum N=512)

    with TileContext(nc) as tc:
        with (
            tc.tile_pool(name="consts", bufs=1) as consts,
            tc.tile_pool(name="wlp", bufs=3) as wlp,
            tc.tile_pool(name="xcp", bufs=8) as xcp,
            tc.tile_pool(name="dp", bufs=3) as dp,
            tc.tile_pool(name="wp", bufs=1) as wp,
            tc.tile_pool(name="op", bufs=3) as op,
            tc.tile_pool(name="pw", bufs=2, space="PSUM") as pwp,
            tc.tile_pool(name="po", bufs=1, space="PSUM") as pop,
        ):
            # three parallel DMA streams (per-queue BW is the limiter):
            # SP + ACT HWDGE queues, and the gpsimd SWDGE queue
            streams = [nc.sync, nc.scalar, nc.gpsimd]

            # constants up front on the gpsimd stream (tiny)
            b0f_t = consts.tile([128, 128], F16, tag="b0f")
            b0r_t = consts.tile([128, 128], F16, tag="b0r")
            b1_t = consts.tile([128, 128], F16, tag="b1")
            nc.gpsimd.dma_start(out=b0f_t[:], in_=b0f[:])
            nc.gpsimd.dma_start(out=b0r_t[:], in_=b0r[:])
            nc.gpsimd.dma_start(out=b1_t[:], in_=b1[:])
            bs_t = []
            for ot, (o0, ow) in enumerate(OTS):
                t = consts.tile([128, 1], F32, tag=f"bs{ot}", name=f"bs{ot}")
                nc.gpsimd.dma_start(out=t[0:ow, :], in_=bsum[o0 : o0 + ow, :])
                bs_t.append(t)

            # weight blocks first (phase A critical path), striped over all 3
            # streams; then x chunks (needed progressively through phase B)
            wl_t = {}
            for c in range(NK // 2):
                wl_t[c] = wlp.tile([128, 2 * WBLK], F16, tag="wl", name=f"wl{c}")
                streams[c % 3].dma_start(
                    out=wl_t[c][:], in_=wpk[:, 2 * WBLK * c : 2 * WBLK * (c + 1)]
                )
            xc_t = {}
            for c in range(NK // 2):
                xc_t[c] = xcp.tile([128, 2 * BC], F16, tag="xc", name=f"xc{c}")
                streams[(NK // 2 + c) % 3].dma_start(
                    out=xc_t[c][:], in_=xpk[:, 2 * BC * c : 2 * BC * (c + 1)]
                )

            # psum: 6 out accumulators (1 bank each) + double-buffered W-prep bank
            po_t = {}
            for ot in range(len(OTS)):
                for h in range(NBH):
                    po_t[ot, h] = pop.tile(
                        [128, 512], F32, tag=f"po{ot}_{h}", name=f"po{ot}_{h}"
                    )

            def wslice(k, which):  # tW half (0) or sW half (1) of S-block k
                base = (k % 2) * WBLK + which * OC
                return wl_t[k // 2][:, base : base + OC]

            def xslice(k, h):
                return xc_t[k // 2][:, (k % 2) * BC + 512 * h : (k % 2) * BC + 512 * (h + 1)]

            # ---- Phase A: build all W_eff blocks (kept resident, 16 x 90KB fp16)
            d_t, w_t = {}, {}
            for k in range(NK + 1):
                if k < NK:
                    d_t[k] = dp.tile([128, OC], F16, tag="d", name=f"d{k}")
                    nc.vector.tensor_tensor(
                        out=d_t[k][:], in0=wslice(k, 0), in1=wslice(k, 1),
                        op=mybir.AluOpType.subtract,
                    )
                if k >= 1:
                    j = k - 1
                    # banded matmuls: pw = b0 @ D_j (+ b1 @ D_{j+1})
                    pw = pwp.tile([128, OC], F32, tag="pw", name=f"pw{j}")
                    b0 = b0f_t if j == 0 else b0r_t
                    last = j == NK - 1
                    nc.tensor.matmul(pw[:], b0[:], d_t[j][:], start=True, stop=last)
                    if not last:
                        nc.tensor.matmul(pw[:], b1_t[:], d_t[j + 1][:], start=False, stop=True)
                    # W_eff^T_j = pw + sWT_j   (PSUM evac on DVE)
                    w_t[j] = wp.tile([128, OC], F16, tag=f"w{j}", name=f"w{j}")
                    nc.vector.tensor_tensor(
                        out=w_t[j][:], in0=pw[:], in1=wslice(j, 1), op=mybir.AluOpType.add
                    )
            # ---- Phase B: one dense, wait-free PE stream of 96 accumulating matmuls
            for k in range(NK):
                for ot, (o0, ow) in enumerate(OTS):
                    for h in range(NBH):
                        nc.tensor.matmul(
                            po_t[ot, h][0:ow, :],
                            w_t[k][:, o0 : o0 + ow],
                            xslice(k, h),
                            start=(k == 0),
                            stop=(k == NK - 1),
                        )
            # epilogue: bias add fused into PSUM evac on ScalarE, then store
            # (stores split across the two HWDGE queues)
            for ot, (o0, ow) in enumerate(OTS):
                for h in range(NBH):
                    osb = op.tile([128, 512], F32, tag="o", name=f"osb{ot}_{h}")
                    nc.scalar.activation(
                        out=osb[0:ow, :], in_=po_t[ot, h][0:ow, :],
                        func=mybir.ActivationFunctionType.Identity,
                        bias=bs_t[ot][0:ow, :],
                    )
                    streams[(ot * NBH + h) % 3].dma_start(
                        out=outT[o0 : o0 + ow, 512 * h : 512 * (h + 1)],
                        in_=osb[0:ow, :],
                    )

    nc.compile()
    return nc


def kernel(x, trend_W, trend_b, seasonal_W, seasonal_b):
    global _nc_cache
    if _nc_cache is None:
        _nc_cache = _build_nc()
    in_maps = build_in_maps(x, trend_W, trend_b, seasonal_W, seasonal_b)
    res = run_bass_kernel_spmd(_nc_cache, in_maps, list(range(NCORES)))
    full = np.empty((O, B), np.float32)
    for i, r in enumerate(res.results):
        g, c = i // NBG, i % NBG
        full[g * OC : (g + 1) * OC, c * BC : (c + 1) * BC] = r["outT"]
    return np.ascontiguousarray(full.T)
